# revision 2
# baseline (speedup 1.0000x reference)
"""KNN learner kernel for Trainium2 (8 NeuronCores, SPMD) — v3.

Algorithm (queries sharded 512/core; support replicated):
1. PE: approx scores q.s - 0.5||s||^2 with a single fp8(e4m3) DoubleRow
   matmul (2x bf16 rate) + one 2-partition bf16 matmul folding in the
   -0.5||s||^2 rows (scaled 256/8; approx err <= ~0.01).
2. Act: PSUM -> SBUF score copies.
3. DVE: top-8 candidates/query via per-half Max8 + combine + full-row
   MaxIndex8 over fp32 scores. (Measured: true argmin ranks <= 4 under
   this approximation over all 4096 queries; K=8 rescue margin.)
4. Pool SWDGE: per-candidate row gather from S_aug[NS, 1032] (fp32
   embedding + -0.5||s||^2 at col 1024).
5. DVE: exact fp32 rescore via scalar_tensor_tensor accumulate against
   qfx (query row with 1.0 at col 1024) -> final argmax among 8 ->
   one-hot dot recovers the global index -> label row gather.

Schedule: software pipeline — rescue(t-1) is emitted between tile t's
score fill and tile t's selection scan, so the DVE rescores tile t-1
(gathers landed during the previous window) while PE/Act fill tile t.
"""

import numpy as np
import ml_dtypes

NS, NQ, D, NCLS = 8192, 4096, 1024, 64
NCORES = 8
QPC = NQ // NCORES          # queries per core (512)
P = 128                     # partitions
KT = D // P                 # k tiles (8)
NCHUNK = 512                # psum chunk (matmul free dim)
CHUNKS = NS // NCHUNK       # 16
QTILES = QPC // P           # 4
K = 8                       # rescued candidates per query
DAUG = 1032                 # augmented row: 1024 emb + s2 + pad
HALF = NS // 2

_E4 = ml_dtypes.float8_e4m3
_BF = ml_dtypes.bfloat16

LAST_RESULT = None


def _build_program():
    import concourse.bass as bass
    import concourse.mybir as mybir
    from concourse import bacc
    from concourse.tile import TileContext

    f32 = mybir.dt.float32
    u32 = mybir.dt.uint32
    fp8 = mybir.dt.float8e4
    bf16 = mybir.dt.bfloat16
    DR = mybir.MatmulPerfMode.DoubleRow
    mult = mybir.AluOpType.mult

    nc = bacc.Bacc()

    q8T = nc.declare_dram_parameter("q8T", [D, QPC], fp8, isOutput=False)
    s8T = nc.declare_dram_parameter("s8T", [D, NS], fp8, isOutput=False)
    s2r = nc.declare_dram_parameter("s2r", [2, NS], bf16, isOutput=False)
    wrow = nc.declare_dram_parameter("wrow", [2, P], bf16, isOutput=False)
    qfx = nc.declare_dram_parameter("qfx", [QPC, DAUG], f32, isOutput=False)
    Sa = nc.declare_dram_parameter("Sa", [NS, DAUG], f32, isOutput=False)
    iota8 = nc.declare_dram_parameter("iota8", [P, 8], f32, isOutput=False)
    labels = nc.declare_dram_parameter("labels", [NS, NCLS], f32, isOutput=False)
    out_lab = nc.declare_dram_parameter("out_lab", [QPC, NCLS], f32, isOutput=True)
    out_idx = nc.declare_dram_parameter("out_idx", [QPC, 1], u32, isOutput=True)

    with TileContext(nc) as tc:
        with (
            tc.tile_pool(name="qres", bufs=1) as qpool,
            tc.tile_pool(name="s8", bufs=CHUNKS) as s8pool,
            tc.tile_pool(name="qfs", bufs=2) as qfpool,
            tc.tile_pool(name="scores", bufs=2) as scpool,
            tc.tile_pool(name="gat", bufs=10) as gpool,
            tc.tile_pool(name="fin", bufs=2) as fpool,
            tc.tile_pool(name="scr", bufs=1) as scrpool,
            tc.tile_pool(name="psum", bufs=8, space="PSUM") as ppool,
        ):
            # --- loads ordered so PE can start ASAP ---
            q8_sb = qpool.tile([P, KT, QPC], fp8, tag="q8")
            nc.sync.dma_start(q8_sb[:], q8T[:].rearrange("(o p) q -> p o q", p=P))
            w_sb = qpool.tile([2, P], bf16, tag="w")
            nc.sync.dma_start(w_sb[:], wrow[:])
            s2r_sb = qpool.tile([2, NS], bf16, tag="s2r")
            nc.sync.dma_start(s2r_sb[:], s2r[:])

            s8T_v = s8T[:].rearrange("(o p) s -> p o s", p=P)
            s8c = []
            for c in range(CHUNKS):
                cs = slice(c * NCHUNK, (c + 1) * NCHUNK)
                t8 = s8pool.tile([P, KT, NCHUNK], fp8, tag="s8c", name=f"s8c{c}")
                nc.sync.dma_start(t8[:], s8T_v[:, :, cs])
                s8c.append(t8)

            iota_f = qpool.tile([P, 8], f32, tag="iotf")
            nc.sync.dma_start(iota_f[:], iota8[:])

            qfx_v = qfx[:].rearrange("(t p) d -> t p d", p=P)

            pend = None  # rescue state of the previous tile

            def emit_rescue(st):
                (t, gidx, Gs, qfx_t) = st
                # exact scores of the 8 candidates, s2 included via col 1024
                ex = fpool.tile([P, K], f32, tag="ex", name=f"ex{t}")
                for j in range(K):
                    scr = scrpool.tile([P, DAUG], f32, tag="scr", name=f"scr{t}_{j}")
                    nc.vector.scalar_tensor_tensor(
                        out=scr[:], in0=Gs[j][:], scalar=1.0, in1=qfx_t[:],
                        op0=mult, op1=mult, accum_out=ex[:, j : j + 1],
                    )
                m1 = fpool.tile([P, 8], f32, tag="m1", name=f"m1{t}")
                nc.vector.max(out=m1[:], in_=ex[:])
                pos = fpool.tile([P, 8], u32, tag="pos", name=f"pos{t}")
                nc.vector.max_index(out=pos[:], in_max=m1[:], in_values=ex[:])
                posf = fpool.tile([P, 1], f32, tag="posf", name=f"posf{t}")
                nc.gpsimd.tensor_copy(out=posf[:], in_=pos[:, :1])
                oh = fpool.tile([P, 8], f32, tag="oh", name=f"oh{t}")
                nc.gpsimd.tensor_scalar(
                    out=oh[:], in0=iota_f[:], scalar1=posf[:], scalar2=None,
                    op0=mybir.AluOpType.is_equal,
                )
                gidxf = fpool.tile([P, 8], f32, tag="gidxf", name=f"gidxf{t}")
                nc.gpsimd.tensor_copy(out=gidxf[:], in_=gidx[:])
                scr2 = fpool.tile([P, 8], f32, tag="scr2", name=f"scr2{t}")
                bidxf = fpool.tile([P, 1], f32, tag="bidxf", name=f"bidxf{t}")
                nc.vector.scalar_tensor_tensor(
                    out=scr2[:], in0=oh[:], scalar=1.0, in1=gidxf[:],
                    op0=mult, op1=mult, accum_out=bidxf[:],
                )
                bidx = fpool.tile([P, 1], u32, tag="bidx", name=f"bidx{t}")
                nc.gpsimd.tensor_copy(out=bidx[:], in_=bidxf[:])

                lab = fpool.tile([P, NCLS], f32, tag="lab", name=f"lab{t}")
                nc.gpsimd.indirect_dma_start(
                    out=lab[:], out_offset=None, in_=labels[:],
                    in_offset=bass.IndirectOffsetOnAxis(ap=bidx[:], axis=0),
                )
                rs = slice(t * P, (t + 1) * P)
                nc.sync.dma_start(out_lab[rs, :], lab[:])
                nc.sync.dma_start(out_idx[rs, :], bidx[:])

            for t in range(QTILES):
                qs = slice(t * P, (t + 1) * P)
                qfx_t = qfpool.tile([P, DAUG], f32, tag="qfx", name=f"qfx{t}")
                nc.sync.dma_start(qfx_t[:], qfx_v[t])
                scores = scpool.tile([P, NS], f32, tag="scores", name=f"sc{t}")
                for c in range(CHUNKS):
                    cs = slice(c * NCHUNK, (c + 1) * NCHUNK)
                    ps = ppool.tile([P, NCHUNK], f32, tag="ps")
                    for g in range(KT // 2):
                        nc.tensor.matmul(
                            ps[:],
                            lhsT=q8_sb[:, 2 * g : 2 * g + 2, qs],
                            rhs=s8c[c][:, 2 * g : 2 * g + 2, :],
                            start=(g == 0), stop=False, perf_mode=DR,
                        )
                    nc.tensor.matmul(
                        ps[:], lhsT=w_sb[:], rhs=s2r_sb[:, cs],
                        start=False, stop=True,
                    )
                    nc.scalar.copy(out=scores[:, cs], in_=ps[:])

                # rescue(t-1) on DVE while PE/Act fill tile t's scores
                if pend is not None:
                    emit_rescue(pend)

                # selection(t)
                catv = fpool.tile([P, 16], f32, tag="catv", name=f"catv{t}")
                nc.vector.max(out=catv[:, :8], in_=scores[:, :HALF])
                nc.vector.max(out=catv[:, 8:], in_=scores[:, HALF:])
                gmax = fpool.tile([P, 8], f32, tag="gmax", name=f"gmax{t}")
                nc.vector.max(out=gmax[:], in_=catv[:])
                gidx = fpool.tile([P, 8], u32, tag="gidx", name=f"gidx{t}")
                nc.vector.max_index(out=gidx[:], in_max=gmax[:], in_values=scores[:])

                Gs = []
                for j in range(K):
                    Gj = gpool.tile([P, DAUG], f32, tag="G", name=f"G{t}_{j}")
                    nc.gpsimd.indirect_dma_start(
                        out=Gj[:], out_offset=None, in_=Sa[:],
                        in_offset=bass.IndirectOffsetOnAxis(ap=gidx[:, j : j + 1], axis=0),
                    )
                    Gs.append(Gj)
                pend = (t, gidx, Gs, qfx_t)

            emit_rescue(pend)

    nc.finalize()
    return nc


def _prep_host(support_embeddings, query_embeddings, support_labels_onehot):
    S = np.asarray(support_embeddings, dtype=np.float32)
    Q = np.asarray(query_embeddings, dtype=np.float32)
    L = np.ascontiguousarray(np.asarray(support_labels_onehot, dtype=np.float32))

    s8T = np.ascontiguousarray(S.astype(_E4).T)                  # [D, NS]
    s2 = -0.5 * (S.astype(np.float64) ** 2).sum(axis=1)          # [NS]
    r0 = (s2 / 256.0).astype(_BF)
    r1 = ((s2 - 256.0 * r0.astype(np.float64)) / 8.0).astype(_BF)
    s2r = np.ascontiguousarray(np.stack([r0, r1]))                # [2, NS]
    wrow = np.empty((2, P), dtype=_BF)
    wrow[0, :] = _BF(256.0)
    wrow[1, :] = _BF(8.0)

    Sa = np.zeros((NS, DAUG), dtype=np.float32)
    Sa[:, :D] = S
    Sa[:, D] = s2.astype(np.float32)
    iota8 = np.broadcast_to(np.arange(8, dtype=np.float32)[None, :], (P, 8)).copy()
    shared = dict(s8T=s8T, s2r=s2r, wrow=wrow, Sa=Sa, iota8=iota8, labels=L)

    in_maps = []
    for c in range(NCORES):
        qs = slice(c * QPC, (c + 1) * QPC)
        Qc = Q[qs]
        qfx = np.zeros((QPC, DAUG), dtype=np.float32)
        qfx[:, :D] = Qc
        qfx[:, D] = 1.0
        in_maps.append(dict(
            q8T=np.ascontiguousarray(Qc.astype(_E4).T),
            qfx=qfx,
            **shared,
        ))
    return in_maps


def kernel(support_embeddings, query_embeddings, support_labels_onehot):
    global LAST_RESULT
    from concourse.bass_utils import run_bass_kernel_spmd

    in_maps = _prep_host(support_embeddings, query_embeddings, support_labels_onehot)
    nc = _build_program()
    res = run_bass_kernel_spmd(nc, in_maps, list(range(NCORES)))
    LAST_RESULT = res
    out = np.concatenate([res.results[c]["out_lab"] for c in range(NCORES)], axis=0)
    return np.ascontiguousarray(out.astype(np.float32))
